# revision 1
# baseline (speedup 1.0000x reference)
"""MDTA (Restormer Multi-DConv-head Transposed Attention) Bass kernel for 8x TRN2 cores.

Strategy (per core = one batch image, data-parallel over B=8):
  x [192, 128, 128] ->
  1x1 conv (GEMM, fp16 operands / fp32 accum) ->
  depthwise 3x3 (9 accumulating diagonal matmuls on TensorE) ->
  channel-transposed attention:
     per 128-spatial chunk: PE-transpose q,k -> [q|k] gram matmuls accumulate
     [96,96] per head in PSUM (diag gives the L2 norms for free) ->
     softmax on [48,48] tiles -> attn @ v -> 1x1 proj -> out fp32.
All big matmuls in fp16 (1 cyc/row on PE); accumulation fp32 in PSUM.
"""
import sys

for _p in ("/opt/trn_rl_repo", "/root/.axon_site/_ro/trn_rl_repo"):
    if _p not in sys.path:
        sys.path.insert(0, _p)

import numpy as np
import ml_dtypes  # noqa: F401

import concourse.bass as bass
from concourse import bacc, mybir
import concourse.tile as tile
from concourse.bass_utils import run_bass_kernel_spmd

F16 = mybir.dt.bfloat16   # PE-native: 2 cols/cycle; fp16 measured 5x slower
F32 = mybir.dt.float32

B, C, HH, WW = 8, 192, 128, 128
N = HH * WW              # 16384
HEADS, HD = 4, 48
C3 = 3 * C               # 576
CS = [128, 128, 128, 128, 64]          # channel chunks of 576
CSTART = [0, 128, 256, 384, 512]
RSLAB = 16               # output rows per slab
NSLAB = HH // RSLAB      # 16
NT = N // 512            # 32 pass-2 col tiles

_CACHE = {}
USE_DMA_TRANSPOSE = True


def _row_groups(jlo, jhi):
    """Groups of <=4 rows covering [jlo, jhi)."""
    out = []
    j = jlo
    while j < jhi:
        out.append((j, min(j + 4, jhi)))
        j = min(j + 4, jhi)
    return out


def build_program(inv_temp: float):
    nc = bacc.Bacc("TRN2", target_bir_lowering=False, debug=False, num_devices=8)

    x16 = nc.dram_tensor("x16", [C, HH, WW], F16, kind="ExternalInput").ap()
    wpwa_d = nc.dram_tensor("wpwa", [128, C3], F16, kind="ExternalInput").ap()
    wpwb_d = nc.dram_tensor("wpwb", [64, C3], F16, kind="ExternalInput").ap()
    wd_d = nc.dram_tensor("wdiag", [128, 5, 9, 128], F16, kind="ExternalInput").ap()
    wj1_d = nc.dram_tensor("wpjT1", [96, C], F16, kind="ExternalInput").ap()
    wj2_d = nc.dram_tensor("wpjT2", [96, C], F16, kind="ExternalInput").ap()
    id16_d = nc.dram_tensor("ident16", [128, 128], F16, kind="ExternalInput").ap()
    id32_d = nc.dram_tensor("ident32", [96, 96], F32, kind="ExternalInput").ap()
    bmask_d = nc.dram_tensor("bmask", [96, 96], F32, kind="ExternalInput").ap()
    out_d = nc.dram_tensor("out", [C, N], F32, kind="ExternalOutput").ap()

    from contextlib import ExitStack
    with tile.TileContext(nc) as tc:
        with tc.tile_pool(name="res", bufs=1) as res, \
             tc.tile_pool(name="xp", bufs=2) as xp, \
             tc.tile_pool(name="qpre", bufs=1) as qpre, \
             tc.tile_pool(name="qk", bufs=2) as qkp, \
             tc.tile_pool(name="qkt", bufs=32) as qktp, \
             tc.tile_pool(name="sm", bufs=1) as sm:
            p1 = ExitStack()
            pwps = p1.enter_context(tc.tile_pool(name="pwps", bufs=2, space="PSUM"))
            dwps = p1.enter_context(tc.tile_pool(name="dwps", bufs=4, space="PSUM"))
            gps = p1.enter_context(tc.tile_pool(name="gps", bufs=1, space="PSUM"))
            tpps = None if USE_DMA_TRANSPOSE else p1.enter_context(
                tc.tile_pool(name="tpps", bufs=2, space="PSUM"))

            # --- resident weights/identities ---
            wpa = res.tile([128, C3], F16, tag="wpa")
            wpb = res.tile([64, C3], F16, tag="wpb")
            wd = res.tile([128, 5, 9, 128], F16, tag="wd")
            wj1 = res.tile([96, C], F16, tag="wj1")
            wj2 = res.tile([96, C], F16, tag="wj2")
            id16 = res.tile([128, 128], F16, tag="id16")
            id32 = res.tile([96, 96], F32, tag="id32")
            bmask = res.tile([96, 96], F32, tag="bmask")
            nc.sync.dma_start(wpa[:], wpwa_d[:])
            nc.sync.dma_start(wpb[:], wpwb_d[:])
            nc.sync.dma_start(wd[:], wd_d[:])
            nc.sync.dma_start(wj1[:], wj1_d[:])
            nc.sync.dma_start(wj2[:], wj2_d[:])
            nc.sync.dma_start(id16[:], id16_d[:])
            nc.sync.dma_start(id32[:], id32_d[:])
            nc.sync.dma_start(bmask[:], bmask_d[:])

            # v, fp16, resident: v16a = channels 0:96 (head pair 0), v16b = 96:192
            v16a = res.tile([96, N], F16, tag="v16a")
            v16b = res.tile([96, N], F16, tag="v16b")
            # gram accumulators, one bank per head pair: cols [qq | G | kk]
            g_t = [gps.tile([96, 288], F32, tag=f"g{p}", name=f"g{p}") for p in range(2)]

            # ---- pass 1: pw + dw; gram software-pipelined one slab behind ----
            def emit_gram(qkts, s_of):
                for rr in range(RSLAB):
                    qkt = qkts[rr]
                    qkt3 = qkt.rearrange("p (two c) -> p two c", two=2)
                    first = s_of == 0 and rr == 0
                    last = s_of == NSLAB - 1 and rr == RSLAB - 1
                    for p in range(2):
                        qpair = qkt[:, 96 * p:96 * p + 96]
                        kpair = qkt[:, 192 + 96 * p:288 + 96 * p]
                        qk2 = qkt3[:, :, 96 * p:96 * p + 96]
                        nc.tensor.matmul(g_t[p][:, 0:192], qpair, qk2,
                                         start=first, stop=False)
                        nc.tensor.matmul(g_t[p][:, 192:288], kpair, kpair,
                                         start=False, stop=last)

            pend = None
            for s in range(NSLAB):
                r0 = RSLAB * s
                jlo = 1 if s == 0 else 0
                jhi = RSLAB + 1 if s == NSLAB - 1 else RSLAB + 2

                xa = xp.tile([128, RSLAB + 2, 128], F16, tag="xa")
                xb = xp.tile([64, RSLAB + 2, 128], F16, tag="xb")
                nc.sync.dma_start(xa[:, jlo:jhi, :], x16[0:128, r0 - 1 + jlo:r0 - 1 + jhi, :])
                nc.sync.dma_start(xb[:, jlo:jhi, :], x16[128:192, r0 - 1 + jlo:r0 - 1 + jhi, :])

                qp = [qpre.tile([CS[m], RSLAB + 2, 130], F16, tag=f"qp{m}", name=f"qp{m}_{s}") for m in range(5)]
                for m in range(5):
                    nc.gpsimd.memset(qp[m][:, :, 0:1], 0.0)
                    nc.gpsimd.memset(qp[m][:, :, 129:130], 0.0)
                    if s == 0:
                        nc.gpsimd.memset(qp[m][:, 0:1, :], 0.0)
                    if s == NSLAB - 1:
                        nc.gpsimd.memset(qp[m][:, RSLAB + 1:RSLAB + 2, :], 0.0)

                # pointwise conv
                for (ja, jb) in _row_groups(jlo, jhi):
                    nr = jb - ja
                    for m in range(5):
                        cs, c0 = CS[m], CSTART[m]
                        ps = pwps.tile([cs, nr * 128], F32, tag="pw")
                        nc.tensor.matmul(ps[:], wpa[:, c0:c0 + cs], xa[:, ja:jb, :],
                                         start=True, stop=False)
                        nc.tensor.matmul(ps[:], wpb[:, c0:c0 + cs], xb[:, ja:jb, :],
                                         start=False, stop=True)
                        nc.scalar.copy(qp[m][:, ja:jb, 1:129], ps[:])

                # gram matmuls for the previous slab (transposes long since done)
                if pend is not None:
                    emit_gram(pend, s - 1)
                    pend = None

                # depthwise conv; qk tiles collect q,k channels for transposes
                qk = [qkp.tile([128, RSLAB, 128], F16, tag=f"qk{i}", name=f"qk{i}_{s}") for i in range(3)]
                for m in range(5):
                    cs = CS[m]
                    dps2 = [dwps.tile([cs, 512], F32, tag="dw", name=f"dw_{s}_{m}_{g}")
                            for g in range(RSLAB // 4)]
                    for t in range(9):
                        dy, dx = t // 3, t % 3
                        for g in range(RSLAB // 4):
                            nc.tensor.matmul(
                                dps2[g][:], wd[0:cs, m, t, 0:cs],
                                qp[m][:, 4 * g + dy:4 * g + dy + 4, dx:dx + 128],
                                start=(t == 0), stop=(t == 8))
                    for g in range(RSLAB // 4):
                        dps = dps2[g]
                        ncol = r0 * 128 + g * 512
                        if m < 3:
                            nc.vector.tensor_copy(qk[m][:, 4 * g:4 * g + 4, :], dps[:])
                        elif m == 3:
                            nc.vector.tensor_copy(v16a[0:96, ncol:ncol + 512], dps[0:96, :])
                            nc.vector.tensor_copy(v16b[0:32, ncol:ncol + 512], dps[96:128, :])
                        else:
                            nc.vector.tensor_copy(v16b[32:64, ncol:ncol + 512], dps[0:32, :])
                            nc.vector.tensor_copy(v16b[64:96, ncol:ncol + 512], dps[32:64, :])

                # transposes for this slab (consumed next iteration)
                qkts = []
                for rr in range(RSLAB):
                    qkt = qktp.tile([128, 384], F16, tag="qkt", name=f"qkt_{s}_{rr}")
                    if USE_DMA_TRANSPOSE:
                        for i in range(3):
                            nc.sync.dma_start_transpose(qkt[:, 128 * i:128 * i + 128],
                                                        qk[i][:, rr, :])
                    else:
                        tp = tpps.tile([128, 384], F16, tag="tp", name=f"tp_{s}_{rr}")
                        for i in range(3):
                            nc.tensor.transpose(tp[:, 128 * i:128 * i + 128],
                                                qk[i][:, rr, :], id16[:])
                        nc.scalar.copy(qkt[:], tp[:])
                    qkts.append(qkt[:])
                pend = qkts

            emit_gram(pend, NSLAB - 1)
            pend = None

            # ------------- softmax (head-pair [96,96] with block mask) -------------
            gs = [sm.tile([96, 288], F32, tag=f"gs{p}", name=f"gs{p}") for p in range(2)]
            for p in range(2):
                nc.scalar.copy(gs[p][:], g_t[p][:])
            p1.close()
            with tc.tile_pool(name="smps", bufs=2, space="PSUM") as smps:
                bd = [sm.tile([96, 96], F16, tag=f"bd{p}", name=f"bd{p}") for p in range(2)]
                dq = [sm.tile([96, 96], F32, tag=f"dq{p}", name=f"dq{p}") for p in range(2)]
                dk = [sm.tile([96, 96], F32, tag=f"dk{p}", name=f"dk{p}") for p in range(2)]
                sqq = [sm.tile([96, 1], F32, tag=f"sqq{p}", name=f"sqq{p}") for p in range(2)]
                skk = [sm.tile([96, 1], F32, tag=f"skk{p}", name=f"skk{p}") for p in range(2)]
                rq = [sm.tile([96, 1], F32, tag=f"rq{p}", name=f"rq{p}") for p in range(2)]
                rk = [sm.tile([96, 1], F32, tag=f"rk{p}", name=f"rk{p}") for p in range(2)]
                rqT = [sm.tile([1, 96], F32, tag=f"rqT{p}", name=f"rqT{p}") for p in range(2)]
                rkT = [sm.tile([1, 96], F32, tag=f"rkT{p}", name=f"rkT{p}") for p in range(2)]
                logit = [sm.tile([96, 96], F32, tag=f"lg{p}", name=f"lg{p}") for p in range(2)]
                nmax = [sm.tile([96, 1], F32, tag=f"nm{p}", name=f"nm{p}") for p in range(2)]
                ex = [sm.tile([96, 96], F32, tag=f"ex{p}", name=f"ex{p}") for p in range(2)]
                rs = [sm.tile([96, 1], F32, tag=f"rs{p}", name=f"rs{p}") for p in range(2)]
                aw = [sm.tile([96, 96], F16, tag=f"aw{p}", name=f"aw{p}") for p in range(2)]

                for p in range(2):   # ||q||^2, ||k||^2 from gram diagonals
                    nc.vector.tensor_mul(dq[p][:], gs[p][:, 0:96], id32[:])
                    nc.vector.reduce_sum(sqq[p][:], dq[p][:], axis=mybir.AxisListType.X)
                    nc.vector.tensor_mul(dk[p][:], gs[p][:, 192:288], id32[:])
                    nc.vector.reduce_sum(skk[p][:], dk[p][:], axis=mybir.AxisListType.X)
                for p in range(2):   # sqrt(s/temp) = ||.||/sqrt(temp)
                    nc.scalar.activation(rq[p][:], sqq[p][:],
                                         mybir.ActivationFunctionType.Sqrt,
                                         scale=float(inv_temp))
                    nc.scalar.activation(rk[p][:], skk[p][:],
                                         mybir.ActivationFunctionType.Sqrt,
                                         scale=float(inv_temp))
                for p in range(2):
                    nc.vector.reciprocal(rq[p][:], rq[p][:])
                    nc.vector.reciprocal(rk[p][:], rk[p][:])
                for p in range(2):
                    tq = smps.tile([1, 96], F32, tag="rt", name=f"tq{p}")
                    nc.tensor.transpose(tq[:], rq[p][:], id32[:])
                    nc.vector.tensor_copy(rqT[p][:], tq[:])
                    tk = smps.tile([1, 96], F32, tag="rt", name=f"tk{p}")
                    nc.tensor.transpose(tk[:], rk[p][:], id32[:])
                    nc.vector.tensor_copy(rkT[p][:], tk[:])
                for p in range(2):   # logits = G * (rq x rk)
                    ops = smps.tile([96, 96], F32, tag="outer", name=f"op{p}")
                    nc.tensor.matmul(ops[:], rqT[p][0:1, :], rkT[p][0:1, :],
                                     start=True, stop=True)
                    nc.vector.tensor_mul(logit[p][:], gs[p][:, 96:192], ops[:])
                for p in range(2):
                    nc.vector.reduce_max(nmax[p][:], logit[p][:], axis=mybir.AxisListType.X)
                    nc.vector.tensor_scalar_mul(nmax[p][:], nmax[p][:], -1.0)
                for p in range(2):
                    nc.scalar.activation(ex[p][:], logit[p][:],
                                         mybir.ActivationFunctionType.Exp,
                                         bias=nmax[p][:])
                for p in range(2):   # mask cross-head blocks, normalize rows
                    nc.vector.tensor_mul(ex[p][:], ex[p][:], bmask[:])
                    nc.vector.reduce_sum(rs[p][:], ex[p][:], axis=mybir.AxisListType.X)
                    nc.vector.reciprocal(rs[p][:], rs[p][:])
                    nc.vector.tensor_scalar_mul(aw[p][:], ex[p][:], rs[p][:])
                for p in range(2):   # bd = aw^T (block-diagonal attention, fp16)
                    aps = smps.tile([96, 96], F16, tag="awT", name=f"aps{p}")
                    nc.tensor.transpose(aps[:], aw[p][:], id16[0:96, 0:96])
                    nc.scalar.copy(bd[p][:], aps[:])

            # ---------------- pass 2: attn @ v + proj ----------------
            with tc.tile_pool(name="avps", bufs=2, space="PSUM") as avps, \
                 tc.tile_pool(name="pops", bufs=2, space="PSUM") as pops, \
                 tc.tile_pool(name="sav", bufs=2) as savp, \
                 tc.tile_pool(name="osb", bufs=2) as osbp:
                for nt in range(NT):
                    col = 512 * nt
                    av1 = avps.tile([96, 512], F32, tag="av1")
                    nc.tensor.matmul(av1[:], bd[0][:], v16a[:, col:col + 512],
                                     start=True, stop=True)
                    av2 = avps.tile([96, 512], F32, tag="av2")
                    nc.tensor.matmul(av2[:], bd[1][:], v16b[:, col:col + 512],
                                     start=True, stop=True)
                    sa1 = savp.tile([96, 512], F16, tag="sa1")
                    nc.scalar.copy(sa1[:], av1[:])
                    sa2 = savp.tile([96, 512], F16, tag="sa2")
                    nc.vector.tensor_copy(sa2[:], av2[:])
                    po1 = pops.tile([128, 512], F32, tag="po1")
                    nc.tensor.matmul(po1[:], wj1[:, 0:128], sa1[:], start=True, stop=False)
                    nc.tensor.matmul(po1[:], wj2[:, 0:128], sa2[:], start=False, stop=True)
                    po2 = pops.tile([64, 512], F32, tag="po2")
                    nc.tensor.matmul(po2[:], wj1[:, 128:192], sa1[:], start=True, stop=False)
                    nc.tensor.matmul(po2[:], wj2[:, 128:192], sa2[:], start=False, stop=True)
                    o1 = osbp.tile([128, 512], F32, tag="o1")
                    nc.vector.tensor_copy(o1[:], po1[:])
                    o2 = osbp.tile([64, 512], F32, tag="o2")
                    nc.scalar.copy(o2[:], po2[:])
                    nc.sync.dma_start(out_d[0:128, col:col + 512], o1[:])
                    nc.sync.dma_start(out_d[128:192, col:col + 512], o2[:])

    nc.compile()
    return nc


def _host_inputs(x, w_pw, w_dw, w_proj):
    """Build the per-core DRAM input maps (weights shared across cores)."""
    f16 = ml_dtypes.bfloat16
    wpwT = np.ascontiguousarray(w_pw.T).astype(f16)        # [192, 576]
    shared = {
        "wpwa": wpwT[0:128],
        "wpwb": wpwT[128:192],
        "wpjT1": np.ascontiguousarray(w_proj.T[0:96]).astype(f16),
        "wpjT2": np.ascontiguousarray(w_proj.T[96:192]).astype(f16),
        "ident16": np.eye(128, dtype=f16),
        "ident32": np.eye(96, dtype=np.float32),
        "bmask": np.kron(np.eye(2, dtype=np.float32), np.ones((48, 48), np.float32)),
    }
    wd9 = w_dw.reshape(C3, 9).astype(np.float32)
    wdiag = np.zeros((128, 5, 9, 128), np.float32)
    for m in range(5):
        cs, c0 = CS[m], CSTART[m]
        for t in range(9):
            wdiag[np.arange(cs), m, t, np.arange(cs)] = wd9[c0:c0 + cs, t]
    shared["wdiag"] = wdiag.astype(f16)

    maps = []
    for b in range(B):
        m = dict(shared)
        m["x16"] = x[b].astype(f16)
        maps.append(m)
    return maps


def kernel(x, w_pw, w_dw, w_proj, temperature, num_heads):
    x = np.asarray(x)
    w_pw = np.asarray(w_pw)
    w_dw = np.asarray(w_dw)
    w_proj = np.asarray(w_proj)
    temp = float(np.asarray(temperature))
    assert int(num_heads) == HEADS and x.shape == (B, C, HH, WW)

    key = ("prog", temp)
    if key not in _CACHE:
        _CACHE[key] = build_program(1.0 / temp)
    nc = _CACHE[key]

    in_maps = _host_inputs(x, w_pw, w_dw, w_proj)
    res = run_bass_kernel_spmd(nc, in_maps, core_ids=list(range(8)))
    out = np.stack([res.results[b]["out"].reshape(C, HH, WW) for b in range(B)])
    return out.astype(np.float32)


if __name__ == "__main__":
    rng = np.random.default_rng(0)
    x = rng.standard_normal((B, C, HH, WW), dtype=np.float32)
    w_pw = rng.standard_normal((C3, C), dtype=np.float32) * C ** -0.5
    w_dw = rng.standard_normal((C3, 1, 3, 3), dtype=np.float32) / 3.0
    w_proj = rng.standard_normal((C, C), dtype=np.float32) * C ** -0.5
    y = kernel(x, w_pw, w_dw, w_proj, np.float32((C / HEADS) ** -0.5), HEADS)
    print("out", y.shape, y.dtype, float(np.abs(y).max()))



# revision 46
# speedup vs baseline: 2.0144x; 2.0144x over previous
"""MDTA (Restormer transposed attention) Bass kernel for 8x TRN2 cores.

Data-parallel: one batch image per core. Per-core pipeline (all bf16 matmuls,
fp32 PSUM accumulation):

  pass 1 (8 row-slabs of 16 rows, software-pipelined):
    PE:   pointwise 1x1 conv (GEMM, K=192 in 2 passes)
    ACT/Pool: PSUM->SBUF drains into halo'd qkv slab tiles
    depthwise 3x3, split by output rows between engines:
      PE rows 0..R-1: 8 diagonal matmuls/psum + center tap absorbed into the
                      Pool scalar_tensor_tensor drain
      DVE rows R..15: 9 tensor_scalar (4x mode) + 8 tensor_tensor adds
    DMA:  batched tiled transposes of q,k slab -> [spatial, ch] tiles
    PE:   per-head gram [q_h|k_h]^T [q_h|k_h] accumulated in PSUM over slabs
  softmax: norms from gram diagonal, per-head scaled logits, pair softmax
  M^T = (W_proj @ blockdiag(attn))^T via one matmul per head pair
  pass 2: out = M @ v on PE (proj fused with attention), drains, DMA out.
"""
import sys

for _p in ("/opt/trn_rl_repo", "/root/.axon_site/_ro/trn_rl_repo"):
    if _p not in sys.path:
        sys.path.insert(0, _p)

import numpy as np
import ml_dtypes  # noqa: F401

import concourse.bass as bass
from concourse import bacc, mybir
import concourse.tile as tile
from concourse.bass_utils import run_bass_kernel_spmd

F16 = mybir.dt.bfloat16
F32 = mybir.dt.float32

B, C, HH, WW = 8, 192, 128, 128
N = HH * WW              # 16384
HEADS, HD = 4, 48
C3 = 3 * C               # 576
CS = [128, 128, 128, 128, 64]
CSTART = [0, 128, 256, 384, 512]
RS = 16                  # rows per slab
NS = HH // RS            # 8 slabs
RPE = 8                  # dw rows 0..RPE-1 on PE, RPE..15 on DVE (multiple of 4)
CENTER = 4               # tap index absorbed into the STT drain
NT = N // 512            # pass-2 column tiles

_CACHE = {}

TAPS = [(t // 3, t % 3) for t in range(9)]


def build_program(inv_temp: float):
    nc = bacc.Bacc("TRN2", target_bir_lowering=False, debug=False, num_devices=8)

    x16 = nc.dram_tensor("x16", [C, HH, WW], F16, kind="ExternalInput").ap()
    wpwa_d = nc.dram_tensor("wpwa", [128, C3], F16, kind="ExternalInput").ap()
    wpwb_d = nc.dram_tensor("wpwb", [64, C3], F16, kind="ExternalInput").ap()
    wd_d = nc.dram_tensor("wdiag", [128, 5, 9, 128], F16, kind="ExternalInput").ap()
    wdv_d = nc.dram_tensor("wdvec", [128, 45], F32, kind="ExternalInput").ap()
    wj1_d = nc.dram_tensor("wpjT1", [96, C], F16, kind="ExternalInput").ap()
    wj2_d = nc.dram_tensor("wpjT2", [96, C], F16, kind="ExternalInput").ap()
    id96_d = nc.dram_tensor("ident96", [96, 96], F32, kind="ExternalInput").ap()
    bmask_d = nc.dram_tensor("bmask", [96, 96], F32, kind="ExternalInput").ap()
    out_d = nc.dram_tensor("out", [C, N], F32, kind="ExternalOutput").ap()
    v3_d = nc.dram_tensor("v3scratch", [128, HH, 128], F16, kind="Internal").ap()
    v4_d = nc.dram_tensor("v4scratch", [64, HH, 128], F16, kind="Internal").ap()

    from contextlib import ExitStack
    with tile.TileContext(nc) as tc:
        with tc.tile_pool(name="res", bufs=1) as res, \
             tc.tile_pool(name="sm", bufs=1) as sm:
            p0 = ExitStack()
            xp = p0.enter_context(tc.tile_pool(name="xp", bufs=2))
            qpool = p0.enter_context(tc.tile_pool(name="qp", bufs=3))
            qkdp = p0.enter_context(tc.tile_pool(name="qkd", bufs=2))
            vstp = p0.enter_context(tc.tile_pool(name="vst", bufs=1))
            qktp = p0.enter_context(tc.tile_pool(name="qkt", bufs=2))
            tmpp = p0.enter_context(tc.tile_pool(name="tmp", bufs=2))
            p1 = ExitStack()
            pwps = p1.enter_context(tc.tile_pool(name="pwps", bufs=4, space="PSUM"))
            dwps = p1.enter_context(tc.tile_pool(name="dwps", bufs=2, space="PSUM"))
            gps = p1.enter_context(tc.tile_pool(name="gps", bufs=1, space="PSUM"))

            # resident weights
            wpa = res.tile([128, C3], F16, tag="wpa")
            wpb = res.tile([64, C3], F16, tag="wpb")
            wd = res.tile([128, 5, 9, 128], F16, tag="wd")
            wdv = res.tile([128, 45], F32, tag="wdv")
            wj1 = res.tile([96, C], F16, tag="wj1")
            wj2 = res.tile([96, C], F16, tag="wj2")
            id96 = res.tile([96, 96], F32, tag="id96")
            bmask = res.tile([96, 96], F32, tag="bmask")
            nc.sync.dma_start(wpa[:], wpwa_d[:])
            nc.sync.dma_start(wpb[:], wpwb_d[:])

            # gram accumulators, one per head pair: cols [qq(96) qk(96) kk(96)]
            G = [gps.tile([96, 288], F32, tag=f"G{p}", name=f"G{p}")
                 for p in range(2)]

            Qs = {}    # slab -> list of 5 qkv tiles [cs, 18, 130]
            first_g = [True, True]

            def emit_pw(s):
                """pointwise conv for slab s -> Qs[s] rows 1..16."""
                xa = xp.tile([128, RS, 128], F16, tag="xa", name=f"xa{s}")
                xb = xp.tile([64, RS, 128], F16, tag="xb", name=f"xb{s}")
                nc.sync.dma_start(xa[:], x16[0:128, RS * s:RS * s + RS, :])
                nc.sync.dma_start(xb[:], x16[128:192, RS * s:RS * s + RS, :])
                q = [qpool.tile([CS[m], RS + 2, 130], F16, tag=f"q{m}",
                                name=f"q{m}_{s}") for m in range(5)]
                Qs[s] = q
                for m in range(5):
                    # column halo zeros
                    nc.gpsimd.memset(q[m][:, :, 0:1], 0.0)
                    nc.gpsimd.memset(q[m][:, :, 129:130], 0.0)
                for g in range(RS // 4):
                    for m in range(5):
                        cs, c0 = CS[m], CSTART[m]
                        ps = pwps.tile([cs, 512], F32, tag="pw",
                                       name=f"pw_{s}_{g}_{m}")
                        nc.tensor.matmul(ps[:], wpa[:, c0:c0 + cs],
                                         xa[:, 4 * g:4 * g + 4, :],
                                         start=True, stop=False)
                        nc.tensor.matmul(ps[:], wpb[:, c0:c0 + cs],
                                         xb[:, 4 * g:4 * g + 4, :],
                                         start=False, stop=True)
                        dst = q[m][:, 4 * g + 1:4 * g + 5, 1:129]
                        nc.scalar.copy(dst, ps[:])

            def emit_boundary(s):
                """fill bottom halo of slab s and top halo of slab s+1."""
                for m in range(5):
                    if s + 1 < NS:
                        nc.gpsimd.tensor_copy(Qs[s][m][:, RS + 1:RS + 2, :],
                                              Qs[s + 1][m][:, 1:2, :])
                        nc.gpsimd.tensor_copy(Qs[s + 1][m][:, 0:1, :],
                                              Qs[s][m][:, RS:RS + 1, :])
                    else:
                        nc.gpsimd.memset(Qs[s][m][:, RS + 1:RS + 2, :], 0.0)

            def emit_dw(s):
                """depthwise 3x3 for slab s; writes qk slab dests + v tiles."""
                q = Qs[s]
                qkds = [qkdp.tile([128, RS, 128], F16, tag=f"qkd{m}",
                                  name=f"qkd{m}_{s}") for m in range(3)]
                vst3 = vstp.tile([128, RS, 128], F16, tag="vst3", name=f"vst3_{s}")
                vst4 = vstp.tile([64, RS, 128], F16, tag="vst4", name=f"vst4_{s}")
                for m in range(5):
                    cs = CS[m]
                    if m < 3:
                        dst = qkds[m]
                    elif m == 3:
                        dst = vst3
                    else:
                        dst = vst4
                    dget = lambda rlo, rhi, d=dst: d[:, rlo:rhi, :]
                    # PE route: rows 0..RPE-1 (all 9 taps, ACT drains)
                    for g in range(RPE // 4):
                        ps = dwps.tile([cs, 512], F32, tag="dw",
                                       name=f"dw_{s}_{m}_{g}")
                        for t in range(9):
                            ty, tx = TAPS[t]
                            nc.tensor.matmul(
                                ps[:], wd[0:cs, m, t, 0:cs],
                                q[m][:, 4 * g + ty:4 * g + ty + 4, tx:tx + 128],
                                start=(t == 0), stop=(t == 8))
                        nc.scalar.copy(dget(4 * g, 4 * g + 4), ps[:])
                    # DVE route: rows RPE..15, chains of 5/2/2 taps with the
                    # two chain merges on Pool
                    if RPE < RS:
                        nr = RS - RPE

                        def src(t):
                            ty, tx = TAPS[t]
                            return q[m][:, RPE + ty:RPE + ty + nr, tx:tx + 128]

                        def sc(t):
                            return wdv[0:cs, 9 * m + t:9 * m + t + 1]

                        acc = dget(RPE, RS)
                        tmp0 = tmpp.tile([cs, nr, 128], F16, tag="tmp0",
                                         name=f"tmp0_{m}_{s}")
                        tmpB = tmpp.tile([cs, nr, 128], F16, tag="tmpB",
                                         name=f"tmpB_{m}_{s}")
                        tmpC = tmpp.tile([cs, nr, 128], F16, tag="tmpC",
                                         name=f"tmpC_{m}_{s}")
                        nc.vector.tensor_scalar_mul(acc, src(0), sc(0))
                        for t in (1, 2, 3, 4):
                            nc.vector.tensor_scalar_mul(tmp0[:], src(t), sc(t))
                            nc.vector.tensor_tensor(acc, acc, tmp0[:],
                                                    mybir.AluOpType.add)
                        nc.vector.tensor_scalar_mul(tmpB[:], src(5), sc(5))
                        nc.vector.tensor_scalar_mul(tmp0[:], src(6), sc(6))
                        nc.vector.tensor_tensor(tmpB[:], tmpB[:], tmp0[:],
                                                mybir.AluOpType.add)
                        nc.vector.tensor_scalar_mul(tmpC[:], src(7), sc(7))
                        nc.vector.tensor_scalar_mul(tmp0[:], src(8), sc(8))
                        nc.vector.tensor_tensor(tmpC[:], tmpC[:], tmp0[:],
                                                mybir.AluOpType.add)
                        nc.gpsimd.tensor_tensor(acc, acc, tmpB[:],
                                                mybir.AluOpType.add)
                        nc.gpsimd.tensor_tensor(acc, acc, tmpC[:],
                                                mybir.AluOpType.add)
                nc.sync.dma_start(v3_d[:, RS * s:RS * s + RS, :], vst3[:])
                nc.sync.dma_start(v4_d[:, RS * s:RS * s + RS, :], vst4[:])
                return qkds

            def emit_gram(s, qkds):
                """transpose q,k slab (two half-slabs) and accumulate grams."""
                # channels are host-permuted so pair p's [q-pair | k-pair] is
                # the contiguous col range 192p:192p+192 of the transposed tile
                for half in range(2):
                    qkt = qktp.tile([128, RS // 2, 384], F16, tag="qkt",
                                    name=f"qkt_{s}_{half}")
                    for m in range(3):
                        nc.sync.dma_start_transpose(
                            qkt[:, :, 128 * m:128 * m + 128],
                            qkds[m][:, RS // 2 * half:RS // 2 * (half + 1), :])
                    for r in range(RS // 2):
                        for p in range(2):
                            qpair = qkt[:, r, 192 * p:192 * p + 96]
                            kpair = qkt[:, r, 192 * p + 96:192 * p + 192]
                            qk = qkt[:, r, 192 * p:192 * p + 192]
                            last = (s == NS - 1) and (half == 1) and (r == RS // 2 - 1)
                            nc.tensor.matmul(G[p][:, 0:192], qpair, qk,
                                             start=first_g[p], stop=False)
                            nc.tensor.matmul(G[p][:, 192:288], kpair, kpair,
                                             start=False, stop=last)
                            first_g[p] = False

            # ---- pass 1, software-pipelined: pw(s+2) ahead, gram lags 1 ----
            emit_pw(0)
            nc.sync.dma_start(wd[:], wd_d[:])
            nc.sync.dma_start(wdv[:], wdv_d[:])
            nc.sync.dma_start(wj1[:], wj1_d[:])
            nc.sync.dma_start(wj2[:], wj2_d[:])
            nc.sync.dma_start(id96[:], id96_d[:])
            nc.sync.dma_start(bmask[:], bmask_d[:])
            # preload activation tables used by the softmax tail
            actwarm = sm.tile([1, 1], F32, tag="actwarm")
            nc.gpsimd.memset(actwarm[:], 1.0)
            nc.scalar.activation(actwarm[:], actwarm[:],
                                 mybir.ActivationFunctionType.Sqrt)
            nc.scalar.activation(actwarm[:], actwarm[:],
                                 mybir.ActivationFunctionType.Exp)
            for m in range(5):
                nc.gpsimd.memset(Qs[0][m][:, 0:1, :], 0.0)
            emit_pw(1)
            pend = None
            for s in range(NS):
                emit_boundary(s)
                qkds = emit_dw(s)
                if s + 2 < NS:
                    emit_pw(s + 2)
                if pend is not None:
                    emit_gram(s - 1, pend)
                pend = qkds
            emit_gram(NS - 1, pend)

            # ---------------- softmax + M^T (baseline pair flow) ----------------
            gs = [sm.tile([96, 288], F32, tag=f"gs{p}", name=f"gs{p}")
                  for p in range(2)]
            for p in range(2):
                nc.scalar.copy(gs[p][:], G[p][:])
            p1.close()
            MTa = sm.tile([128, C], F16, tag="MTa")
            MTb = sm.tile([64, C], F16, tag="MTb")
            with tc.tile_pool(name="smps", bufs=2, space="PSUM") as smps:
                dq = [sm.tile([96, 96], F32, tag=f"dq{p}", name=f"dq{p}") for p in range(2)]
                dk = [sm.tile([96, 96], F32, tag=f"dk{p}", name=f"dk{p}") for p in range(2)]
                sqq = [sm.tile([96, 1], F32, tag=f"sqq{p}", name=f"sqq{p}") for p in range(2)]
                skk = [sm.tile([96, 1], F32, tag=f"skk{p}", name=f"skk{p}") for p in range(2)]
                rq = [sm.tile([96, 1], F32, tag=f"rq{p}", name=f"rq{p}") for p in range(2)]
                rk = [sm.tile([96, 1], F32, tag=f"rk{p}", name=f"rk{p}") for p in range(2)]
                rqT = [sm.tile([1, 96], F32, tag=f"rqT{p}", name=f"rqT{p}") for p in range(2)]
                rkT = [sm.tile([1, 96], F32, tag=f"rkT{p}", name=f"rkT{p}") for p in range(2)]
                logit = [sm.tile([96, 96], F32, tag=f"lg{p}", name=f"lg{p}") for p in range(2)]
                nmax = [sm.tile([96, 1], F32, tag=f"nm{p}", name=f"nm{p}") for p in range(2)]
                ex = [sm.tile([96, 96], F32, tag=f"ex{p}", name=f"ex{p}") for p in range(2)]
                rs_ = [sm.tile([96, 1], F32, tag=f"rs{p}", name=f"rs{p}") for p in range(2)]
                aw = [sm.tile([96, 96], F16, tag=f"aw{p}", name=f"aw{p}") for p in range(2)]

                for p in range(2):   # ||q||^2, ||k||^2 from gram diagonals
                    nc.vector.tensor_tensor(dq[p][:], gs[p][:, 0:96], id96[:],
                                            mybir.AluOpType.mult)
                    nc.vector.tensor_reduce(sqq[p][:], dq[p][:],
                                            mybir.AxisListType.X,
                                            mybir.AluOpType.add)
                    nc.vector.tensor_tensor(dk[p][:], gs[p][:, 192:288], id96[:],
                                            mybir.AluOpType.mult)
                    nc.vector.tensor_reduce(skk[p][:], dk[p][:],
                                            mybir.AxisListType.X,
                                            mybir.AluOpType.add)
                for p in range(2):   # 1/sqrt(s/temp)
                    nc.scalar.activation(rq[p][:], sqq[p][:],
                                         mybir.ActivationFunctionType.Sqrt,
                                         scale=float(inv_temp))
                    nc.scalar.activation(rk[p][:], skk[p][:],
                                         mybir.ActivationFunctionType.Sqrt,
                                         scale=float(inv_temp))
                for p in range(2):
                    nc.vector.reciprocal(rq[p][:], rq[p][:])
                    nc.vector.reciprocal(rk[p][:], rk[p][:])
                for p in range(2):
                    tq = smps.tile([1, 96], F32, tag="rt", name=f"tq{p}")
                    nc.tensor.transpose(tq[:], rq[p][:], id96[:])
                    nc.vector.tensor_copy(rqT[p][:], tq[:])
                    tk = smps.tile([1, 96], F32, tag="rt", name=f"tk{p}")
                    nc.tensor.transpose(tk[:], rk[p][:], id96[:])
                    nc.vector.tensor_copy(rkT[p][:], tk[:])
                for p in range(2):   # logits = qk-gram * (rq x rk)
                    op = smps.tile([96, 96], F32, tag="outer", name=f"op{p}")
                    nc.tensor.matmul(op[:], rqT[p][0:1, :], rkT[p][0:1, :],
                                     start=True, stop=True)
                    nc.vector.tensor_tensor(logit[p][:], gs[p][:, 96:192], op[:],
                                            mybir.AluOpType.mult)
                for p in range(2):
                    nc.vector.tensor_reduce(nmax[p][:], logit[p][:],
                                            mybir.AxisListType.X,
                                            mybir.AluOpType.max)
                    nc.vector.tensor_scalar_mul(nmax[p][:], nmax[p][:], -1.0)
                for p in range(2):
                    nc.scalar.activation(ex[p][:], logit[p][:],
                                         mybir.ActivationFunctionType.Exp,
                                         bias=nmax[p][:])
                for p in range(2):   # mask cross-head blocks, normalize rows
                    nc.vector.tensor_tensor(ex[p][:], ex[p][:], bmask[:],
                                            mybir.AluOpType.mult)
                    nc.vector.tensor_reduce(rs_[p][:], ex[p][:],
                                            mybir.AxisListType.X,
                                            mybir.AluOpType.add)
                    nc.vector.reciprocal(rs_[p][:], rs_[p][:])
                    nc.vector.tensor_scalar_mul(aw[p][:], ex[p][:], rs_[p][:])
                # M^T = (W_proj @ A)^T, assembled as [128,192]+[64,192] bf16
                for p in range(2):
                    wj = wj1 if p == 0 else wj2
                    mt = smps.tile([96, C], F32, tag="mt", name=f"mt{p}")
                    nc.tensor.matmul(mt[:], aw[p][:], wj[:], start=True, stop=True)
                    if p == 0:
                        nc.vector.tensor_copy(MTa[0:96, :], mt[:])
                    else:
                        nc.vector.tensor_copy(MTa[96:128, :], mt[0:32, :])
                        nc.vector.tensor_copy(MTb[0:32, :], mt[32:64, :])
                        nc.vector.tensor_copy(MTb[32:64, :], mt[64:96, :])

            # ---------------- pass 2: out = M @ v ----------------
            p0.close()
            with tc.tile_pool(name="mvps", bufs=4, space="PSUM") as mvps, \
                 tc.tile_pool(name="vin", bufs=3) as vinp, \
                 tc.tile_pool(name="osb", bufs=3) as osbp:
                for g4 in range(NT // 4):
                    rg = 16 * g4
                    vi3 = vinp.tile([128, 16, 128], F16, tag="vi3", name=f"vi3_{g4}")
                    vi4 = vinp.tile([64, 16, 128], F16, tag="vi4", name=f"vi4_{g4}")
                    nc.sync.dma_start(vi3[:], v3_d[:, rg:rg + 16, :])
                    nc.sync.dma_start(vi4[:], v4_d[:, rg:rg + 16, :])
                    o1 = osbp.tile([128, 2048], F32, tag="o1", name=f"o1_{g4}")
                    o2 = osbp.tile([64, 2048], F32, tag="o2", name=f"o2_{g4}")
                    for k in range(4):
                        r0 = 4 * k
                        p1_ = mvps.tile([128, 512], F32, tag="mv1")
                        nc.tensor.matmul(p1_[:], MTa[:, 0:128],
                                         vi3[:, r0:r0 + 4, :], start=True, stop=False)
                        nc.tensor.matmul(p1_[:], MTb[:, 0:128],
                                         vi4[:, r0:r0 + 4, :], start=False, stop=True)
                        p2_ = mvps.tile([64, 512], F32, tag="mv2")
                        nc.tensor.matmul(p2_[:], MTa[:, 128:192],
                                         vi3[:, r0:r0 + 4, :], start=True, stop=False)
                        nc.tensor.matmul(p2_[:], MTb[:, 128:192],
                                         vi4[:, r0:r0 + 4, :], start=False, stop=True)
                        if k % 2 == 0:
                            nc.scalar.copy(o1[:, 512 * k:512 * k + 512], p1_[:])
                            nc.scalar.copy(o2[:, 512 * k:512 * k + 512], p2_[:])
                        else:
                            nc.vector.tensor_copy(o1[:, 512 * k:512 * k + 512], p1_[:])
                            nc.vector.tensor_copy(o2[:, 512 * k:512 * k + 512], p2_[:])
                    nc.sync.dma_start(out_d[0:128, 2048 * g4:2048 * g4 + 2048], o1[:])
                    nc.sync.dma_start(out_d[128:192, 2048 * g4:2048 * g4 + 2048], o2[:])

    nc.compile()
    return nc


def _qk_perm():
    """qkv channel permutation: pair p's [q-pair | k-pair] contiguous; v as-is."""
    perm = np.empty(C3, np.int64)
    for p in range(2):
        perm[192 * p:192 * p + 96] = 96 * p + np.arange(96)          # q pair
        perm[192 * p + 96:192 * p + 192] = C + 96 * p + np.arange(96)  # k pair
    perm[2 * C:] = np.arange(2 * C, C3)                               # v
    return perm


def _host_inputs(x, w_pw, w_dw, w_proj):
    f16 = ml_dtypes.bfloat16
    perm = _qk_perm()
    w_pw = w_pw[perm]
    w_dw = w_dw[perm]
    wpwT = np.ascontiguousarray(w_pw.T).astype(f16)        # [192, 576]
    wd9 = w_dw.reshape(C3, 9).astype(np.float32)
    wdiag = np.zeros((128, 5, 9, 128), np.float32)
    wdv = np.zeros((128, 45), np.float32)
    for m in range(5):
        cs, c0 = CS[m], CSTART[m]
        for t in range(9):
            wdiag[np.arange(cs), m, t, np.arange(cs)] = wd9[c0:c0 + cs, t]
            wdv[0:cs, 9 * m + t] = wd9[c0:c0 + cs, t]
    shared = {
        "wpwa": wpwT[0:128],
        "wpwb": wpwT[128:192],
        "wdiag": wdiag.astype(f16),
        "wdvec": wdv,
        "wpjT1": np.ascontiguousarray(w_proj.T[0:96]).astype(f16),
        "wpjT2": np.ascontiguousarray(w_proj.T[96:192]).astype(f16),
        "ident96": np.eye(96, dtype=np.float32),
        "bmask": np.kron(np.eye(2, dtype=np.float32), np.ones((48, 48), np.float32)),
    }
    maps = []
    for b in range(B):
        m = dict(shared)
        m["x16"] = x[b].astype(f16)
        maps.append(m)
    return maps


def kernel(x, w_pw, w_dw, w_proj, temperature, num_heads):
    x = np.asarray(x)
    w_pw = np.asarray(w_pw)
    w_dw = np.asarray(w_dw)
    w_proj = np.asarray(w_proj)
    temp = float(np.asarray(temperature))
    assert int(num_heads) == HEADS and x.shape == (B, C, HH, WW)

    key = ("prog", temp)
    if key not in _CACHE:
        _CACHE[key] = build_program(1.0 / temp)
    nc = _CACHE[key]

    in_maps = _host_inputs(x, w_pw, w_dw, w_proj)
    res = run_bass_kernel_spmd(nc, in_maps, core_ids=list(range(8)))
    out = np.stack([res.results[b]["out"].reshape(C, HH, WW) for b in range(B)])
    return out.astype(np.float32)


def _np_reference(x, w_pw, w_dw, w_proj, temperature):
    """numpy oracle for quick checks."""
    b, c, h, w = x.shape
    hd = c // HEADS
    qkv = np.einsum('oc,bchw->bohw', w_pw, x)
    pad = np.pad(qkv, ((0, 0), (0, 0), (1, 1), (1, 1)))
    dw = np.zeros_like(qkv)
    w9 = w_dw.reshape(3 * c, 3, 3)
    for ty in range(3):
        for tx in range(3):
            dw += w9[None, :, ty, tx, None, None] * pad[:, :, ty:ty + h, tx:tx + w]
    q, k, v = np.split(dw, 3, axis=1)
    shp = (b, HEADS, hd, h * w)
    q = q.reshape(shp); k = k.reshape(shp); v = v.reshape(shp)
    q = q / np.maximum(np.linalg.norm(q, axis=-1, keepdims=True), 1e-12)
    k = k / np.maximum(np.linalg.norm(k, axis=-1, keepdims=True), 1e-12)
    attn = np.einsum('bhcn,bhdn->bhcd', q, k) * temperature
    attn = attn - attn.max(-1, keepdims=True)
    attn = np.exp(attn); attn /= attn.sum(-1, keepdims=True)
    out = np.einsum('bhcd,bhdn->bhcn', attn, v).reshape(b, c, h, w)
    return np.einsum('oc,bchw->bohw', w_proj, out)


if __name__ == "__main__":
    rng = np.random.default_rng(0)
    x = rng.standard_normal((B, C, HH, WW), dtype=np.float32)
    w_pw = (rng.standard_normal((C3, C), dtype=np.float32) * C ** -0.5)
    w_dw = (rng.standard_normal((C3, 1, 3, 3), dtype=np.float32) / 3.0)
    w_proj = (rng.standard_normal((C, C), dtype=np.float32) * C ** -0.5)
    temp = np.float32((C / HEADS) ** -0.5)
    y = kernel(x, w_pw, w_dw, w_proj, temp, HEADS)
    ref = _np_reference(x, w_pw, w_dw, w_proj, float(temp))
    scale = np.abs(ref).max()
    err = np.abs(y - ref).max()
    print(f"rel err vs numpy oracle: {err / scale:.4e} (scale {scale:.3f})")
